# revision 16
# baseline (speedup 1.0000x reference)
"""Graph multi-head attention (GNN message passing) on 8 Trainium2 NeuronCores.

Strategy (dst-sharded edge parallelism, zero indirect DMAs) — v2:
  - Host: sort edges by dst, split nodes into 8 contiguous ranges with ~equal
    edge counts. Each core owns all incoming edges of its node range, so the
    per-dst segment softmax is core-local.
  - Host EXPANDS the raw per-edge operands: for every packed edge slot the
    fp16 [key||value] column of its src node, and per virtual row the fp16
    query column of its dst node, plus the per-tile one-hot combine matrix
    and the slot-validity mask. The device projects k/q/v per edge with
    plain matmuls -- every DMA is a large contiguous load.
  - Edges are packed into fixed-width virtual rows (node, up to D_PAD=8
    incoming edges); rows of one node stay inside one 128-row tile and are
    combined with a host-supplied one-hot matmul, PSUM-accumulated over all
    8 slots (denominator columns ride along), then normalized on DVE.
  - Bias folding: bq via a ones row in the q expansion + [Wq.T; bq] rhs;
    bv folds into bo' = bo + Wo bv (sum(alpha)=1); the q.bk score term is
    DROPPED ENTIRELY -- it is constant across all edges of a (dst, head)
    segment, so it cancels in the segment softmax (exact).
  - v stays in HEAD space through aggregation; tiles are processed in
    PAIRS: one 128x128 PE transpose per pair, then a single block-diagonal
    [[Wo.T,0],[0,Wo.T]] matmul projects both tiles, and one ACT copy with
    per-partition bias adds bo' for both. Output is stored feature-major
    ([128, NPAIR*128]); host unshards with fancy indexing.
  - Segment-max subtraction is replaced by a constant exp shift
    (exp(s/4 - 8)); invalid slots get an additive -30000 fp16 mask.
    Degree-0 nodes are fixed up to `bo` on the host.
"""

import os
from contextlib import ExitStack

import numpy as np

N = 100000
E = 1600000
DIM = 64
H = 4
DK = DIM // H
NCORES = 8

D_PAD = 8          # edge slots per virtual row
TC = 8             # 128-row tiles per supertile
NP = TC // 2       # tile pairs per supertile
SBLK = TC * 1024 + TC * 128 + TC * D_PAD  # merged [kv|oh|msk] block cols
MASKV = -30000.0   # additive fp16-safe -inf
ESHIFT = -8.0      # constant exp shift: keeps exp() in fp16 range both ways


def _host_prep(src, dst):
    """Pack edges into per-core tiling metadata (no feature expansion yet)."""
    src = np.asarray(src).astype(np.int64)
    dst = np.asarray(dst).astype(np.int64)
    order = np.argsort(dst, kind="stable")
    ssrc = src[order]
    deg = np.bincount(dst, minlength=N).astype(np.int64)
    cum = np.concatenate([[0], np.cumsum(deg)])

    bounds = [0]
    for c in range(1, NCORES):
        t = round(c * E / NCORES)
        n = int(np.searchsorted(cum, t, side="left"))
        n = min(max(n, bounds[-1] + 1), N - (NCORES - c))
        bounds.append(n)
    bounds.append(N)

    packs = []
    for c in range(NCORES):
        n0, n1 = bounds[c], bounds[c + 1]
        nn = n1 - n0
        d = deg[n0:n1]
        r_n = np.maximum(1, -(-d // D_PAD)).astype(np.int64)
        tile_of = np.empty(nn, np.int64)
        colrow = np.empty(nn, np.int64)
        crow_of = np.empty(nn, np.int64)
        t_id = 0
        rows_in = 0
        nodes_in = 0
        for i in range(nn):
            r = r_n[i]
            if rows_in + r > 128:
                t_id += 1
                rows_in = 0
                nodes_in = 0
            tile_of[i] = t_id
            colrow[i] = rows_in
            crow_of[i] = nodes_in
            rows_in += r
            nodes_in += 1
        packs.append(dict(n0=n0, n1=n1, nn=nn, d=d, r_n=r_n, tile_of=tile_of,
                          colrow=colrow, crow_of=crow_of, nt=t_id + 1))

    NT = -(-max(p["nt"] for p in packs) // TC) * TC
    return packs, ssrc, cum, NT


def _expand_core(p, ssrc, cum, NT, keyT16, valT16, qT16):
    """Build the per-core expanded fp16 operand arrays."""
    n0 = p["n0"]
    nn = p["nn"]
    d, r_n = p["d"], p["r_n"]
    rows_total = NT * 128

    row_node = np.repeat(np.arange(nn), r_n)
    starts = np.concatenate([[0], np.cumsum(r_n)])[:-1]
    row_k = np.arange(len(row_node)) - np.repeat(starts, r_n)
    row_slot = (np.repeat(p["tile_of"], r_n) * 128
                + np.repeat(p["colrow"], r_n) + row_k)
    row_deg = np.clip(np.repeat(d, r_n) - row_k * D_PAD, 0, D_PAD)
    row_e0 = cum[n0 + row_node] + row_k * D_PAD
    j = np.arange(D_PAD)[None, :]
    valid = j < row_deg[:, None]
    eidx = np.minimum(row_e0[:, None] + j, E - 1)
    srcv = ssrc[eidx]

    # kvx[0:64, col]=key.T[src], [64:128]=value.T[src]; col=T*1024+s*128+p
    kvx = np.zeros((128, NT * 1024), np.float16)
    T_of = row_slot // 128
    p_of = row_slot % 128
    cols = T_of[:, None] * 1024 + j * 128 + p_of[:, None]
    cv = cols[valid]
    sv = srcv[valid]
    kvx[0:64, cv] = keyT16[:, sv]
    kvx[64:128, cv] = valT16[:, sv]

    # qx [65, NT*128], ones row for bq folding
    qx = np.zeros((65, NT * 128), np.float16)
    qx[64, :] = 1.0
    qx[0:64, row_slot] = qT16[:, n0 + row_node]

    # additive mask [128, NT*8], col = T*8 + s (no head replication)
    mrow = np.full((rows_total, D_PAD), MASKV, np.float16)
    mrow[row_slot] = np.where(valid, np.float16(0.0), np.float16(MASKV))
    msk = np.ascontiguousarray(
        mrow.reshape(NT, 128, D_PAD).transpose(1, 0, 2).reshape(128, NT * D_PAD))

    # per-row node-column one-hot [128, NT*128] f16 (host-built, DMA'd in)
    crow_slot = np.zeros(rows_total, np.int32)
    crow_slot[row_slot] = np.repeat(p["crow_of"], r_n).astype(np.int32)
    ohx = (crow_slot.reshape(NT, 128)[:, :, None]
           == np.arange(128, dtype=np.int32)[None, None, :])
    oh = np.ascontiguousarray(
        ohx.transpose(1, 0, 2).reshape(128, NT * 128).astype(np.float16))

    # one merged DMA block per supertile: [kv 8192 | oh 1024 | msk 64]
    ST = NT // TC
    datx = np.empty((128, ST, SBLK), np.float16)
    datx[:, :, 0:TC * 1024] = kvx.reshape(128, ST, TC * 1024)
    datx[:, :, TC * 1024:TC * 1152] = oh.reshape(128, ST, TC * 128)
    datx[:, :, TC * 1152:SBLK] = msk.reshape(128, ST, TC * D_PAD)

    return dict(datx=np.ascontiguousarray(datx.reshape(128, ST * SBLK)), qx=qx)


def _build_program(NT):
    import concourse.bass as bass
    import concourse.tile as tile
    from concourse import bacc, mybir

    f32 = mybir.dt.float32
    f16 = mybir.dt.float16
    AO = mybir.AluOpType
    ST = NT // TC

    nc = bacc.Bacc("TRN2", target_bir_lowering=False, debug=False,
                   num_devices=NCORES)

    datxd = nc.dram_tensor("datx", [128, (NT // TC) * SBLK], f16,
                           kind="ExternalInput").ap()
    qx = nc.dram_tensor("qx", [65, NT * 128], f16, kind="ExternalInput").ap()
    wkvd = nc.dram_tensor("wkv", [128, 128], f16, kind="ExternalInput").ap()
    wqd = nc.dram_tensor("wq", [65, DIM], f16, kind="ExternalInput").ap()
    wo2d = nc.dram_tensor("wo2", [128, 128], f16, kind="ExternalInput").ap()
    bo2d = nc.dram_tensor("bo2", [128, 1], f32, kind="ExternalInput").ap()
    comb = nc.dram_tensor("comb", [128, (NT // 2) * 128], f32,
                          kind="ExternalOutput").ap()

    def apx(t, dims, extra_off=0):
        a = t[:]
        return bass.AP(a.tensor, a.offset + extra_off, [list(a.ap[0])] + dims)

    with tile.TileContext(nc) as tc, ExitStack() as ctx, \
            nc.allow_low_precision("fp16 edge softmax within 2e-2 tolerance"):
        consts = ctx.enter_context(tc.tile_pool(name="consts", bufs=1))
        ld = ctx.enter_context(tc.tile_pool(name="ld", bufs=4))
        work = ctx.enter_context(tc.tile_pool(name="work", bufs=3))
        qw = ctx.enter_context(tc.tile_pool(name="qw", bufs=2))
        scw = ctx.enter_context(tc.tile_pool(name="scw", bufs=4))
        adexp = ctx.enter_context(tc.tile_pool(name="adexp", bufs=2 * NP + 2))
        pstp = ctx.enter_context(tc.tile_pool(name="pstp", bufs=2, space="PSUM"))
        qpsp = ctx.enter_context(tc.tile_pool(name="qpsp", bufs=2, space="PSUM"))
        cpsp = ctx.enter_context(tc.tile_pool(name="cpsp", bufs=2, space="PSUM"))

        from concourse.masks import make_identity

        wkv_sb = consts.tile([128, 128], f16)
        nc.sync.dma_start(wkv_sb[:], wkvd[:, :])
        wq_sb = consts.tile([65, DIM], f16)
        nc.sync.dma_start(wq_sb[:], wqd[:, :])
        wo2_sb = consts.tile([128, 128], f16)
        nc.sync.dma_start(wo2_sb[:], wo2d[:, :])
        bo2_sb = consts.tile([128, 1], f32)
        nc.sync.dma_start(bo2_sb[:], bo2d[:, :])
        ident = consts.tile([128, 128], f16)
        make_identity(nc, ident[:])
        esh = consts.tile([128, 1], f32)
        nc.vector.memset(esh[:], ESHIFT)

        prev = None

        def emit_loads(st):
            datld = ld.tile([128, SBLK], f16, tag="datld")
            nc.sync.dma_start(datld[:], datxd[:, st * SBLK:(st + 1) * SBLK])
            qld = ld.tile([65, TC * 128], f16, tag="qld")
            nc.sync.dma_start(qld[:], qx[:, st * TC * 128:(st + 1) * TC * 128])
            return datld, qld

        def emit_A(st, datld, qld):
            kvld = datld  # kv cols [0 : TC*1024]
            OHOFF = TC * 1024
            MOFF = TC * 1152

            # q~ projection, all TC tiles into one PSUM bank, one f16 copy-out
            qps = qpsp.tile([128, TC, DIM], f32, space="PSUM", tag="qps")
            for t in range(TC):
                nc.tensor.matmul(out=qps[:, t, :],
                                 lhsT=qld[:, t * 128:(t + 1) * 128],
                                 rhs=wq_sb[:], start=True, stop=True)
            q16 = qw.tile([128, TC * DIM], f16, tag="q16")
            nc.scalar.copy(q16[:], qps[:])

            adex_l = []
            for u in range(NP):
                sco = scw.tile([128, 2, D_PAD, H], f16, tag="sco")
                adex = adexp.tile([128, 2, D_PAD, DIM + H], f16, tag="adex")
                vtp = work.tile([128, 2, D_PAD * DIM], f16, tag="vt16")
                for tp in range(2):
                    t = 2 * u + tp
                    # per-edge [k^ || v^] projection: 8 slots -> PSUM
                    pst = pstp.tile([128, D_PAD, 128], f32, space="PSUM",
                                    tag="pst")
                    for sl in range(D_PAD):
                        nc.tensor.matmul(
                            out=pst[:, sl, :],
                            lhsT=kvld[:, (t * D_PAD + sl) * 128:
                                      (t * D_PAD + sl + 1) * 128],
                            rhs=wkv_sb[:], start=True, stop=True)
                    # scores: prod = k^ * q~ (slot-bcast), reduce per head
                    prod = work.tile([128, D_PAD, DIM], f16, tag="prod")
                    nc.vector.tensor_tensor(
                        out=prod[:],
                        in0=apx(pst, [[128, D_PAD], [1, DIM]]),
                        in1=apx(q16, [[0, D_PAD], [1, DIM]], extra_off=t * DIM),
                        op=AO.mult)
                    nc.vector.tensor_reduce(
                        out=sco[:, tp],
                        in_=apx(prod, [[DK, D_PAD * H], [1, DK]]),
                        axis=mybir.AxisListType.X, op=AO.add)
                    # v^ to SBUF f16 for the GpSimd weighting
                    nc.scalar.copy(
                        vtp[:, tp], apx(pst, [[128, D_PAD], [1, DIM]],
                                        extra_off=DIM))
                # mask both tiles at once
                nc.vector.tensor_tensor(
                    out=sco[:], in0=sco[:],
                    in1=apx(datld, [[1, 2 * D_PAD], [0, H]],
                            extra_off=MOFF + 2 * u * D_PAD),
                    op=AO.add)
                # exp for both tiles straight into adex cols [64:68]
                nc.scalar.activation(
                    out=apx(adex, [[DIM + H, 2 * D_PAD], [1, H]],
                            extra_off=DIM),
                    in_=sco[:],
                    func=mybir.ActivationFunctionType.Exp,
                    scale=1.0 / np.sqrt(DK), bias=esh[:])
                # adex[., 0:64] = v^ * exp for both tiles in one op
                nc.gpsimd.tensor_tensor(
                    out=apx(adex, [[(DIM + H) * D_PAD, 2], [DIM + H, D_PAD],
                                   [1, DIM]]),
                    in0=vtp[:],
                    in1=apx(adex, [[(DIM + H) * D_PAD, 2], [DIM + H, D_PAD],
                                   [1, H], [0, DK]], extra_off=DIM),
                    op=AO.mult)
                adex_l.append(adex)
            return dict(st=st, adex=adex_l, datld=datld)

        def emit_B(state):
            st = state["st"]
            datld = state["datld"]
            OHOFF = TC * 1024
            osbq = scw.tile([128, NP * 128], f32, tag="osbq")

            def b_comb(u):
                adex = state["adex"][u]
                # one PSUM bank per pair: f32 [0:136] = per-tile combines,
                # f16 elems [272:400] = transpose out, f32 [200:328] = Wo out
                mega = cpsp.tile([128, 512], f32, space="PSUM", tag="mega")
                for tp in range(2):
                    t = 2 * u + tp
                    cp = mega[:, tp * 68:(tp + 1) * 68]
                    for sl in range(D_PAD):
                        nc.tensor.matmul(
                            out=cp,
                            lhsT=datld[:, OHOFF + t * 128:OHOFF + (t + 1) * 128],
                            rhs=adex[:, tp, sl, :],
                            start=(sl == 0), stop=(sl == D_PAD - 1))
                return mega

            def b_tail(u, mega):
                meg16 = mega.bitcast(f16)
                rd = scw.tile([128, 2, H], f32, tag="rd")
                nc.vector.reciprocal(
                    rd[:], apx(mega, [[68, 2], [1, H]], extra_off=DIM))
                # clamp inf (empty node columns, den=0) so 0*rd stays 0 --
                # the block-diag Wo matmul would spread 0*inf=NaN otherwise
                nc.vector.tensor_scalar_min(rd[:], rd[:], 1.0e7)
                nrm2 = scw.tile([128, 2, DIM], f16, tag="nrm2")
                nc.vector.tensor_tensor(
                    out=nrm2[:],
                    in0=apx(mega, [[68, 2], [1, DIM]]),
                    in1=apx(rd, [[H, 2], [1, H], [0, DK]]),
                    op=AO.mult)
                # transpose both tiles at once, then block-diag Wo matmul
                tps = meg16[:, 272:400]
                nc.tensor.transpose(out=tps, in_=apx(nrm2, [[1, 128]]),
                                    identity=ident[:])
                nrmT2 = scw.tile([128, 128], f16, tag="nrmT2")
                nc.scalar.copy(nrmT2[:], tps)
                wout = mega[:, 200:328]
                nc.tensor.matmul(out=wout, lhsT=wo2_sb[:], rhs=nrmT2[:],
                                 start=True, stop=True)
                # bias-add fused into the PSUM->SBUF copy
                nc.scalar.activation(
                    out=osbq[:, u * 128:(u + 1) * 128], in_=wout,
                    func=mybir.ActivationFunctionType.Identity,
                    bias=bo2_sb[:], scale=1.0)

            # pairwise: two combines ahead, so the PE's DVE round-trip wait
            # hides under the following combine
            megas = {}
            megas[0] = b_comb(0)
            megas[1] = b_comb(1)
            b_tail(0, megas[0])
            megas[2] = b_comb(2)
            b_tail(1, megas[1])
            megas[3] = b_comb(3)
            b_tail(2, megas[2])
            b_tail(3, megas[3])
            # one batched store per supertile (sync queue, after prefetches)
            nc.sync.dma_start(
                comb[:, st * NP * 128:(st + 1) * NP * 128], osbq[:])

        loads = {}
        loads[0] = emit_loads(0)
        if ST > 1:
            loads[1] = emit_loads(1)
        for st in range(ST):
            if st + 2 < ST:
                loads[st + 2] = emit_loads(st + 2)
            state = emit_A(st, *loads.pop(st))
            if prev is not None:
                emit_B(prev)
            prev = state
        emit_B(prev)

    nc.compile()
    return nc


def kernel(**inputs):
    from concourse.bass_utils import run_bass_kernel_spmd

    query = np.asarray(inputs["query"], np.float32)
    key = np.asarray(inputs["key"], np.float32)
    value = np.asarray(inputs["value"], np.float32)
    src = np.asarray(inputs["src"])
    dst = np.asarray(inputs["dst"])
    Wq = np.asarray(inputs["Wq"], np.float32)
    bq = np.asarray(inputs["bq"], np.float32)
    Wk = np.asarray(inputs["Wk"], np.float32)
    bk = np.asarray(inputs["bk"], np.float32)  # noqa: F841  (cancels in softmax)
    Wv = np.asarray(inputs["Wv"], np.float32)
    bv = np.asarray(inputs["bv"], np.float32)
    Wo = np.asarray(inputs["Wo"], np.float32)
    bo = np.asarray(inputs["bo"], np.float32)

    packs, ssrc, cum, NT = _host_prep(src, dst)
    nc = _build_program(NT)

    keyT16 = np.ascontiguousarray(key.T).astype(np.float16)
    valT16 = np.ascontiguousarray(value.T).astype(np.float16)
    qT16 = np.ascontiguousarray(query.T).astype(np.float16)

    # weight packing with bias folding (the q.bk score term cancels in the
    # per-segment softmax and is dropped; bv folds into bo')
    wkv = np.zeros((128, 128), np.float16)
    wkv[0:64, 0:64] = Wk.T
    wkv[64:128, 64:128] = Wv.T
    wq = np.zeros((65, DIM), np.float16)
    wq[0:64, :] = Wq.T
    wq[64, :] = bq
    wo2 = np.zeros((128, 128), np.float16)
    wo2[0:64, 0:64] = Wo.T
    wo2[64:128, 64:128] = Wo.T
    bo_eff = (bo + Wo @ bv).astype(np.float32)
    bo2 = np.concatenate([bo_eff, bo_eff]).reshape(128, 1).astype(np.float32)

    in_maps = []
    for p in packs:
        ex = _expand_core(p, ssrc, cum, NT, keyT16, valT16, qT16)
        in_maps.append(dict(datx=ex["datx"], qx=ex["qx"],
                            wkv=wkv, wq=wq, wo2=wo2, bo2=bo2))

    trace = bool(int(os.environ.get("KERNEL_TRACE", "0")))
    res = run_bass_kernel_spmd(
        nc, in_maps, core_ids=list(range(NCORES)), trace=trace,
        tmpdir=os.environ.get("KERNEL_TRACE_DIR") or None,
    )
    kernel.last_results = res

    out = np.empty((N, DIM), np.float32)
    for p, r in zip(packs, res.results):
        cols = (p["tile_of"] // 2) * 128 + p["crow_of"]
        half = (p["tile_of"] % 2).astype(bool)
        sel = r["comb"][:, cols]  # [128, nn]
        out[p["n0"]:p["n1"]] = np.where(half[:, None], sel[64:128, :].T,
                                        sel[0:64, :].T)
        z = p["d"] == 0
        if z.any():
            out[p["n0"]:p["n1"]][z] = bo
    return out


# revision 18
# speedup vs baseline: 1.0656x; 1.0656x over previous
"""Graph multi-head attention (GNN message passing) on 8 Trainium2 NeuronCores.

Strategy (dst-sharded edge parallelism, zero indirect DMAs) — v2:
  - Host: sort edges by dst, split nodes into 8 contiguous ranges with ~equal
    edge counts. Each core owns all incoming edges of its node range, so the
    per-dst segment softmax is core-local.
  - Host EXPANDS the raw per-edge operands: for every packed edge slot the
    fp16 [key||value] column of its src node, and per virtual row the fp16
    query column of its dst node, plus the per-tile one-hot combine matrix
    and the slot-validity mask. The device projects k/q/v per edge with
    plain matmuls -- every DMA is a large contiguous load.
  - Edges are packed into fixed-width virtual rows (node, up to D_PAD=8
    incoming edges); rows of one node stay inside one 128-row tile and are
    combined with a host-supplied one-hot matmul, PSUM-accumulated over all
    8 slots (denominator columns ride along), then normalized on DVE.
  - Bias folding: bq via a ones row in the q expansion + [Wq.T; bq] rhs;
    bv folds into bo' = bo + Wo bv (sum(alpha)=1); the q.bk score term is
    DROPPED ENTIRELY -- it is constant across all edges of a (dst, head)
    segment, so it cancels in the segment softmax (exact).
  - v stays in HEAD space through aggregation; tiles are processed in
    PAIRS: one 128x128 PE transpose per pair, then a single block-diagonal
    [[Wo.T,0],[0,Wo.T]] matmul projects both tiles, and one ACT copy with
    per-partition bias adds bo' for both. Output is stored feature-major
    ([128, NPAIR*128]); host unshards with fancy indexing.
  - Segment-max subtraction is replaced by a constant exp shift
    (exp(s/4 - 8)); invalid slots get an additive -30000 fp16 mask.
    Degree-0 nodes are fixed up to `bo` on the host.
"""

import os
from contextlib import ExitStack

import numpy as np

N = 100000
E = 1600000
DIM = 64
H = 4
DK = DIM // H
NCORES = 8

D_PAD = 8          # edge slots per virtual row
TC = 8             # 128-row tiles per supertile
NP = TC // 2       # tile pairs per supertile
SBLK = TC * 1024 + TC * 128 + TC * D_PAD  # merged [kv|oh|msk] block cols
MASKV = -30000.0   # additive fp16-safe -inf
ESHIFT = -8.0      # constant exp shift: keeps exp() in fp16 range both ways


def _host_prep(src, dst):
    """Pack edges into per-core tiling metadata (no feature expansion yet)."""
    src = np.asarray(src).astype(np.int64)
    dst = np.asarray(dst).astype(np.int64)
    order = np.argsort(dst, kind="stable")
    ssrc = src[order]
    deg = np.bincount(dst, minlength=N).astype(np.int64)
    cum = np.concatenate([[0], np.cumsum(deg)])

    bounds = [0]
    for c in range(1, NCORES):
        t = round(c * E / NCORES)
        n = int(np.searchsorted(cum, t, side="left"))
        n = min(max(n, bounds[-1] + 1), N - (NCORES - c))
        bounds.append(n)
    bounds.append(N)

    packs = []
    for c in range(NCORES):
        n0, n1 = bounds[c], bounds[c + 1]
        nn = n1 - n0
        d = deg[n0:n1]
        r_n = np.maximum(1, -(-d // D_PAD)).astype(np.int64)
        tile_of = np.empty(nn, np.int64)
        colrow = np.empty(nn, np.int64)
        crow_of = np.empty(nn, np.int64)
        t_id = 0
        rows_in = 0
        nodes_in = 0
        for i in range(nn):
            r = r_n[i]
            if rows_in + r > 128:
                t_id += 1
                rows_in = 0
                nodes_in = 0
            tile_of[i] = t_id
            colrow[i] = rows_in
            crow_of[i] = nodes_in
            rows_in += r
            nodes_in += 1
        packs.append(dict(n0=n0, n1=n1, nn=nn, d=d, r_n=r_n, tile_of=tile_of,
                          colrow=colrow, crow_of=crow_of, nt=t_id + 1))

    NT = -(-max(p["nt"] for p in packs) // TC) * TC
    return packs, ssrc, cum, NT


def _expand_core(p, ssrc, cum, NT, keyT16, valT16, qT16):
    """Build the per-core expanded fp16 operand arrays."""
    n0 = p["n0"]
    nn = p["nn"]
    d, r_n = p["d"], p["r_n"]
    rows_total = NT * 128

    row_node = np.repeat(np.arange(nn), r_n)
    starts = np.concatenate([[0], np.cumsum(r_n)])[:-1]
    row_k = np.arange(len(row_node)) - np.repeat(starts, r_n)
    row_slot = (np.repeat(p["tile_of"], r_n) * 128
                + np.repeat(p["colrow"], r_n) + row_k)
    row_deg = np.clip(np.repeat(d, r_n) - row_k * D_PAD, 0, D_PAD)
    row_e0 = cum[n0 + row_node] + row_k * D_PAD
    j = np.arange(D_PAD)[None, :]
    valid = j < row_deg[:, None]
    eidx = np.minimum(row_e0[:, None] + j, E - 1)
    srcv = ssrc[eidx]

    # kvx[0:64, col]=key.T[src], [64:128]=value.T[src]; col=T*1024+s*128+p
    kvx = np.zeros((128, NT * 1024), np.float16)
    T_of = row_slot // 128
    p_of = row_slot % 128
    cols = T_of[:, None] * 1024 + j * 128 + p_of[:, None]
    cv = cols[valid]
    sv = srcv[valid]
    kvx[0:64, cv] = keyT16[:, sv]
    kvx[64:128, cv] = valT16[:, sv]

    # qx [65, NT*128], ones row for bq folding
    qx = np.zeros((65, NT * 128), np.float16)
    qx[64, :] = 1.0
    qx[0:64, row_slot] = qT16[:, n0 + row_node]

    # additive mask [128, NT*8], col = T*8 + s (no head replication)
    mrow = np.full((rows_total, D_PAD), MASKV, np.float16)
    mrow[row_slot] = np.where(valid, np.float16(0.0), np.float16(MASKV))
    msk = np.ascontiguousarray(
        mrow.reshape(NT, 128, D_PAD).transpose(1, 0, 2).reshape(128, NT * D_PAD))

    # per-row node-column one-hot [128, NT*128] f16 (host-built, DMA'd in)
    crow_slot = np.zeros(rows_total, np.int32)
    crow_slot[row_slot] = np.repeat(p["crow_of"], r_n).astype(np.int32)
    ohx = (crow_slot.reshape(NT, 128)[:, :, None]
           == np.arange(128, dtype=np.int32)[None, None, :])
    oh = np.ascontiguousarray(
        ohx.transpose(1, 0, 2).reshape(128, NT * 128).astype(np.float16))

    # one merged DMA block per supertile: [kv 8192 | oh 1024 | msk 64]
    ST = NT // TC
    datx = np.empty((128, ST, SBLK), np.float16)
    datx[:, :, 0:TC * 1024] = kvx.reshape(128, ST, TC * 1024)
    datx[:, :, TC * 1024:TC * 1152] = oh.reshape(128, ST, TC * 128)
    datx[:, :, TC * 1152:SBLK] = msk.reshape(128, ST, TC * D_PAD)

    return dict(datx=np.ascontiguousarray(datx.reshape(128, ST * SBLK)), qx=qx)


def _build_program(NT):
    import concourse.bass as bass
    import concourse.tile as tile
    from concourse import bacc, mybir

    f32 = mybir.dt.float32
    f16 = mybir.dt.float16
    AO = mybir.AluOpType
    ST = NT // TC

    nc = bacc.Bacc("TRN2", target_bir_lowering=False, debug=False,
                   num_devices=NCORES)

    datxd = nc.dram_tensor("datx", [128, (NT // TC) * SBLK], f16,
                           kind="ExternalInput").ap()
    qx = nc.dram_tensor("qx", [65, NT * 128], f16, kind="ExternalInput").ap()
    wkvd = nc.dram_tensor("wkv", [128, 128], f16, kind="ExternalInput").ap()
    wqd = nc.dram_tensor("wq", [65, DIM], f16, kind="ExternalInput").ap()
    wo2d = nc.dram_tensor("wo2", [128, 128], f16, kind="ExternalInput").ap()
    bo2d = nc.dram_tensor("bo2", [128, 1], f32, kind="ExternalInput").ap()
    comb = nc.dram_tensor("comb", [128, (NT // 2) * 128], f32,
                          kind="ExternalOutput").ap()

    def apx(t, dims, extra_off=0):
        a = t[:]
        return bass.AP(a.tensor, a.offset + extra_off, [list(a.ap[0])] + dims)

    with tile.TileContext(nc) as tc, ExitStack() as ctx, \
            nc.allow_low_precision("fp16 edge softmax within 2e-2 tolerance"):
        consts = ctx.enter_context(tc.tile_pool(name="consts", bufs=1))
        ld = ctx.enter_context(tc.tile_pool(name="ld", bufs=4))
        work = ctx.enter_context(tc.tile_pool(name="work", bufs=3))
        qw = ctx.enter_context(tc.tile_pool(name="qw", bufs=2))
        scw = ctx.enter_context(tc.tile_pool(name="scw", bufs=4))
        adexp = ctx.enter_context(tc.tile_pool(name="adexp", bufs=2 * NP + 2))
        pstp = ctx.enter_context(tc.tile_pool(name="pstp", bufs=2, space="PSUM"))
        qpsp = ctx.enter_context(tc.tile_pool(name="qpsp", bufs=2, space="PSUM"))
        cpsp = ctx.enter_context(tc.tile_pool(name="cpsp", bufs=2, space="PSUM"))

        from concourse.masks import make_identity

        wkv_sb = consts.tile([128, 128], f16)
        nc.sync.dma_start(wkv_sb[:], wkvd[:, :])
        wq_sb = consts.tile([65, DIM], f16)
        nc.sync.dma_start(wq_sb[:], wqd[:, :])
        wo2_sb = consts.tile([128, 128], f16)
        nc.sync.dma_start(wo2_sb[:], wo2d[:, :])
        bo2_sb = consts.tile([128, 1], f32)
        nc.sync.dma_start(bo2_sb[:], bo2d[:, :])
        ident = consts.tile([128, 128], f16)
        make_identity(nc, ident[:])
        esh = consts.tile([128, 1], f32)
        nc.vector.memset(esh[:], ESHIFT)

        prev = None

        def emit_loads(st):
            datld = ld.tile([128, SBLK], f16, tag="datld")
            nc.sync.dma_start(datld[:], datxd[:, st * SBLK:(st + 1) * SBLK])
            qld = ld.tile([65, TC * 128], f16, tag="qld")
            nc.sync.dma_start(qld[:], qx[:, st * TC * 128:(st + 1) * TC * 128])
            return datld, qld

        def emit_A(st, datld, qld):
            kvld = datld  # kv cols [0 : TC*1024]
            OHOFF = TC * 1024
            MOFF = TC * 1152

            # q~ projection, all TC tiles into one PSUM bank, one f16 copy-out
            qps = qpsp.tile([128, TC, DIM], f32, space="PSUM", tag="qps")
            for t in range(TC):
                nc.tensor.matmul(out=qps[:, t, :],
                                 lhsT=qld[:, t * 128:(t + 1) * 128],
                                 rhs=wq_sb[:], start=True, stop=True)
            q16 = qw.tile([128, TC * DIM], f16, tag="q16")
            nc.scalar.copy(q16[:], qps[:])

            adex_l = []
            for u in range(NP):
                sco = scw.tile([128, 2, D_PAD, H], f16, tag="sco")
                adex = adexp.tile([128, 2, D_PAD, DIM + H], f16, tag="adex")
                vtp = work.tile([128, 2, D_PAD * DIM], f16, tag="vt16")
                for tp in range(2):
                    t = 2 * u + tp
                    # per-edge [k^ || v^] projection: 8 slots -> PSUM
                    pst = pstp.tile([128, D_PAD, 128], f32, space="PSUM",
                                    tag="pst")
                    for sl in range(D_PAD):
                        nc.tensor.matmul(
                            out=pst[:, sl, :],
                            lhsT=kvld[:, (t * D_PAD + sl) * 128:
                                      (t * D_PAD + sl + 1) * 128],
                            rhs=wkv_sb[:], start=True, stop=True)
                    # scores: prod = k^ * q~ (slot-bcast), reduce per head
                    prod = work.tile([128, D_PAD, DIM], f16, tag="prod")
                    nc.vector.tensor_tensor(
                        out=prod[:],
                        in0=apx(pst, [[128, D_PAD], [1, DIM]]),
                        in1=apx(q16, [[0, D_PAD], [1, DIM]], extra_off=t * DIM),
                        op=AO.mult)
                    nc.vector.tensor_reduce(
                        out=sco[:, tp],
                        in_=apx(prod, [[DK, D_PAD * H], [1, DK]]),
                        axis=mybir.AxisListType.X, op=AO.add)
                    # v^ to SBUF f16 for the GpSimd weighting
                    nc.scalar.copy(
                        vtp[:, tp], apx(pst, [[128, D_PAD], [1, DIM]],
                                        extra_off=DIM))
                # mask both tiles at once (GpSimd: SBUF-only op, DVE is hot)
                nc.gpsimd.tensor_tensor(
                    out=sco[:], in0=sco[:],
                    in1=apx(datld, [[1, 2 * D_PAD], [0, H]],
                            extra_off=MOFF + 2 * u * D_PAD),
                    op=AO.add)
                # exp for both tiles straight into adex cols [64:68]
                nc.scalar.activation(
                    out=apx(adex, [[DIM + H, 2 * D_PAD], [1, H]],
                            extra_off=DIM),
                    in_=sco[:],
                    func=mybir.ActivationFunctionType.Exp,
                    scale=1.0 / np.sqrt(DK), bias=esh[:])
                # adex[., 0:64] = v^ * exp for both tiles in one op
                nc.gpsimd.tensor_tensor(
                    out=apx(adex, [[(DIM + H) * D_PAD, 2], [DIM + H, D_PAD],
                                   [1, DIM]]),
                    in0=vtp[:],
                    in1=apx(adex, [[(DIM + H) * D_PAD, 2], [DIM + H, D_PAD],
                                   [1, H], [0, DK]], extra_off=DIM),
                    op=AO.mult)
                adex_l.append(adex)
            return dict(st=st, adex=adex_l, datld=datld)

        def emit_B(state):
            st = state["st"]
            datld = state["datld"]
            OHOFF = TC * 1024
            osbq = scw.tile([128, NP * 128], f32, tag="osbq")

            def b_comb(u):
                adex = state["adex"][u]
                # one PSUM bank per pair: f32 [0:136] = per-tile combines,
                # f16 elems [272:400] = transpose out, f32 [200:328] = Wo out
                mega = cpsp.tile([128, 512], f32, space="PSUM", tag="mega")
                for tp in range(2):
                    t = 2 * u + tp
                    cp = mega[:, tp * 68:(tp + 1) * 68]
                    for sl in range(D_PAD):
                        nc.tensor.matmul(
                            out=cp,
                            lhsT=datld[:, OHOFF + t * 128:OHOFF + (t + 1) * 128],
                            rhs=adex[:, tp, sl, :],
                            start=(sl == 0), stop=(sl == D_PAD - 1))
                return mega

            def b_tail(u, mega):
                meg16 = mega.bitcast(f16)
                rd = scw.tile([128, 2, H], f32, tag="rd")
                nc.vector.reciprocal(
                    rd[:], apx(mega, [[68, 2], [1, H]], extra_off=DIM))
                # nrm = min(rd, 1e7) * cp in one fused op; the clamp keeps
                # empty node columns (den=0, rd=inf) at 0*big=0 -- the
                # block-diag Wo matmul would spread 0*inf=NaN otherwise
                nrm2 = scw.tile([128, 2, DIM], f16, tag="nrm2")
                nc.vector.scalar_tensor_tensor(
                    out=nrm2[:],
                    in0=apx(rd, [[H, 2], [1, H], [0, DK]]),
                    scalar=1.0e7,
                    in1=apx(mega, [[68, 2], [1, DIM]]),
                    op0=AO.min,
                    op1=AO.mult)
                # transpose both tiles at once, then block-diag Wo matmul
                tps = meg16[:, 272:400]
                nc.tensor.transpose(out=tps, in_=apx(nrm2, [[1, 128]]),
                                    identity=ident[:])
                nrmT2 = scw.tile([128, 128], f16, tag="nrmT2")
                nc.scalar.copy(nrmT2[:], tps)
                wout = mega[:, 200:328]
                nc.tensor.matmul(out=wout, lhsT=wo2_sb[:], rhs=nrmT2[:],
                                 start=True, stop=True)
                # bias-add fused into the PSUM->SBUF copy
                nc.scalar.activation(
                    out=osbq[:, u * 128:(u + 1) * 128], in_=wout,
                    func=mybir.ActivationFunctionType.Identity,
                    bias=bo2_sb[:], scale=1.0)

            # pairwise: two combines ahead, so the PE's DVE round-trip wait
            # hides under the following combine
            megas = {}
            megas[0] = b_comb(0)
            megas[1] = b_comb(1)
            b_tail(0, megas[0])
            megas[2] = b_comb(2)
            b_tail(1, megas[1])
            megas[3] = b_comb(3)
            b_tail(2, megas[2])
            b_tail(3, megas[3])
            # one batched store per supertile (sync queue, after prefetches)
            nc.sync.dma_start(
                comb[:, st * NP * 128:(st + 1) * NP * 128], osbq[:])

        loads = {}
        loads[0] = emit_loads(0)
        if ST > 1:
            loads[1] = emit_loads(1)
        for st in range(ST):
            if st + 2 < ST:
                loads[st + 2] = emit_loads(st + 2)
            state = emit_A(st, *loads.pop(st))
            if prev is not None:
                emit_B(prev)
            prev = state
        emit_B(prev)

    nc.compile()
    return nc


def kernel(**inputs):
    from concourse.bass_utils import run_bass_kernel_spmd

    query = np.asarray(inputs["query"], np.float32)
    key = np.asarray(inputs["key"], np.float32)
    value = np.asarray(inputs["value"], np.float32)
    src = np.asarray(inputs["src"])
    dst = np.asarray(inputs["dst"])
    Wq = np.asarray(inputs["Wq"], np.float32)
    bq = np.asarray(inputs["bq"], np.float32)
    Wk = np.asarray(inputs["Wk"], np.float32)
    bk = np.asarray(inputs["bk"], np.float32)  # noqa: F841  (cancels in softmax)
    Wv = np.asarray(inputs["Wv"], np.float32)
    bv = np.asarray(inputs["bv"], np.float32)
    Wo = np.asarray(inputs["Wo"], np.float32)
    bo = np.asarray(inputs["bo"], np.float32)

    packs, ssrc, cum, NT = _host_prep(src, dst)
    nc = _build_program(NT)

    keyT16 = np.ascontiguousarray(key.T).astype(np.float16)
    valT16 = np.ascontiguousarray(value.T).astype(np.float16)
    qT16 = np.ascontiguousarray(query.T).astype(np.float16)

    # weight packing with bias folding (the q.bk score term cancels in the
    # per-segment softmax and is dropped; bv folds into bo')
    wkv = np.zeros((128, 128), np.float16)
    wkv[0:64, 0:64] = Wk.T
    wkv[64:128, 64:128] = Wv.T
    wq = np.zeros((65, DIM), np.float16)
    wq[0:64, :] = Wq.T
    wq[64, :] = bq
    wo2 = np.zeros((128, 128), np.float16)
    wo2[0:64, 0:64] = Wo.T
    wo2[64:128, 64:128] = Wo.T
    bo_eff = (bo + Wo @ bv).astype(np.float32)
    bo2 = np.concatenate([bo_eff, bo_eff]).reshape(128, 1).astype(np.float32)

    in_maps = []
    for p in packs:
        ex = _expand_core(p, ssrc, cum, NT, keyT16, valT16, qT16)
        in_maps.append(dict(datx=ex["datx"], qx=ex["qx"],
                            wkv=wkv, wq=wq, wo2=wo2, bo2=bo2))

    trace = bool(int(os.environ.get("KERNEL_TRACE", "0")))
    res = run_bass_kernel_spmd(
        nc, in_maps, core_ids=list(range(NCORES)), trace=trace,
        tmpdir=os.environ.get("KERNEL_TRACE_DIR") or None,
    )
    kernel.last_results = res

    out = np.empty((N, DIM), np.float32)
    for p, r in zip(packs, res.results):
        cols = (p["tile_of"] // 2) * 128 + p["crow_of"]
        half = (p["tile_of"] % 2).astype(bool)
        sel = r["comb"][:, cols]  # [128, nn]
        out[p["n0"]:p["n1"]] = np.where(half[:, None], sel[64:128, :].T,
                                        sel[0:64, :].T)
        z = p["d"] == 0
        if z.any():
            out[p["n0"]:p["n1"]][z] = bo
    return out


# revision 21
# speedup vs baseline: 1.1524x; 1.0815x over previous
"""Graph multi-head attention (GNN message passing) on 8 Trainium2 NeuronCores.

Strategy (dst-sharded edge parallelism, zero indirect DMAs):
  - Host: sort edges by dst, split nodes into 8 contiguous ranges with ~equal
    edge counts. Each core owns all incoming edges of its node range, so the
    per-dst segment softmax is core-local.
  - Host EXPANDS the raw per-edge operands: for every packed edge slot the
    fp16 [key||value] column of its src node, and per virtual row the fp16
    query column of its dst node, plus the per-tile one-hot combine matrix
    and the slot-validity mask. The device projects k/q/v per edge with
    plain matmuls -- every DMA is a large contiguous load. kv, one-hot, and
    mask are interleaved into ONE dram block per supertile (single load),
    prefetched two supertiles ahead on the sync queue; outputs are staged
    to one batched store per supertile so stores never gate load prefetch.
  - Edges are packed into fixed-width virtual rows (node, up to D_PAD=8
    incoming edges); rows of one node stay inside one 128-row tile and are
    combined with a host-supplied one-hot matmul, PSUM-accumulated over all
    8 slots (denominator columns ride along), then normalized on DVE.
  - Bias folding: bq via a ones row in the q expansion + [Wq.T; bq] rhs;
    bv folds into bo' = bo + Wo bv (sum(alpha)=1); the q.bk score term is
    DROPPED ENTIRELY -- it is constant across all edges of a (dst, head)
    segment, so it cancels in the segment softmax (exact).
  - Engine balance (DVE is the critical engine): scores via one PSUM-read
    multiply + one pair-level reduce on DVE; mask-add on GpSimd; exp on ACT
    (one op per tile pair, written straight into the combine operand); the
    alpha*v weighting on GpSimd from an ACT-copied f16 v^; normalize via a
    fused (min 1e7)(recip)*cp scalar_tensor_tensor (the clamp keeps empty
    node columns at 0 instead of 0*inf=NaN, which the block-diag Wo matmul
    would otherwise spread); q projections batched per supertile.
  - v stays in HEAD space through aggregation; tiles are processed in
    PAIRS: one 128x128 PE transpose per pair, then a single block-diagonal
    [[Wo.T,0],[0,Wo.T]] matmul projects both tiles, and one ACT copy with
    per-partition bias adds bo' for both. Output is stored feature-major
    ([128, NPAIR*128]); host unshards with fancy indexing.
  - Segment-max subtraction is replaced by a constant exp shift
    (exp(s/4 - 8)); invalid slots get an additive -30000 fp16 mask.
    Degree-0 nodes are fixed up to `bo` on the host.
"""

import os
from contextlib import ExitStack

import numpy as np

N = 100000
E = 1600000
DIM = 64
H = 4
DK = DIM // H
NCORES = 8

D_PAD = 8          # edge slots per virtual row
TC = 8             # 128-row tiles per supertile
NP = TC // 2       # tile pairs per supertile
SBLK = TC * 1024 + TC * 128 + TC * D_PAD  # merged [kv|oh|msk] block cols
MASKV = -30000.0   # additive fp16-safe -inf
ESHIFT = -8.0      # constant exp shift: keeps exp() in fp16 range both ways


def _host_prep(src, dst):
    """Pack edges into per-core tiling metadata (no feature expansion yet)."""
    src = np.asarray(src).astype(np.int64)
    dst = np.asarray(dst).astype(np.int64)
    order = np.argsort(dst, kind="stable")
    ssrc = src[order]
    deg = np.bincount(dst, minlength=N).astype(np.int64)
    cum = np.concatenate([[0], np.cumsum(deg)])

    bounds = [0]
    for c in range(1, NCORES):
        t = round(c * E / NCORES)
        n = int(np.searchsorted(cum, t, side="left"))
        n = min(max(n, bounds[-1] + 1), N - (NCORES - c))
        bounds.append(n)
    bounds.append(N)

    packs = []
    for c in range(NCORES):
        n0, n1 = bounds[c], bounds[c + 1]
        nn = n1 - n0
        d = deg[n0:n1]
        r_n = np.maximum(1, -(-d // D_PAD)).astype(np.int64)
        tile_of = np.empty(nn, np.int64)
        colrow = np.empty(nn, np.int64)
        crow_of = np.empty(nn, np.int64)
        t_id = 0
        rows_in = 0
        nodes_in = 0
        for i in range(nn):
            r = r_n[i]
            if rows_in + r > 128:
                t_id += 1
                rows_in = 0
                nodes_in = 0
            tile_of[i] = t_id
            colrow[i] = rows_in
            crow_of[i] = nodes_in
            rows_in += r
            nodes_in += 1
        packs.append(dict(n0=n0, n1=n1, nn=nn, d=d, r_n=r_n, tile_of=tile_of,
                          colrow=colrow, crow_of=crow_of, nt=t_id + 1))

    NT = -(-max(p["nt"] for p in packs) // TC) * TC
    return packs, ssrc, cum, NT


def _expand_core(p, ssrc, cum, NT, keyT16, valT16, qT16):
    """Build the per-core expanded fp16 operand arrays."""
    n0 = p["n0"]
    nn = p["nn"]
    d, r_n = p["d"], p["r_n"]
    rows_total = NT * 128

    row_node = np.repeat(np.arange(nn), r_n)
    starts = np.concatenate([[0], np.cumsum(r_n)])[:-1]
    row_k = np.arange(len(row_node)) - np.repeat(starts, r_n)
    row_slot = (np.repeat(p["tile_of"], r_n) * 128
                + np.repeat(p["colrow"], r_n) + row_k)
    row_deg = np.clip(np.repeat(d, r_n) - row_k * D_PAD, 0, D_PAD)
    row_e0 = cum[n0 + row_node] + row_k * D_PAD
    j = np.arange(D_PAD)[None, :]
    valid = j < row_deg[:, None]
    eidx = np.minimum(row_e0[:, None] + j, E - 1)
    srcv = ssrc[eidx]

    # kvx[0:64, col]=key.T[src], [64:128]=value.T[src]; col=T*1024+s*128+p
    kvx = np.zeros((128, NT * 1024), np.float16)
    T_of = row_slot // 128
    p_of = row_slot % 128
    cols = T_of[:, None] * 1024 + j * 128 + p_of[:, None]
    cv = cols[valid]
    sv = srcv[valid]
    kvx[0:64, cv] = keyT16[:, sv]
    kvx[64:128, cv] = valT16[:, sv]

    # qx [65, NT*128], ones row for bq folding
    qx = np.zeros((65, NT * 128), np.float16)
    qx[64, :] = 1.0
    qx[0:64, row_slot] = qT16[:, n0 + row_node]

    # additive mask [128, NT*8], col = T*8 + s (no head replication)
    mrow = np.full((rows_total, D_PAD), MASKV, np.float16)
    mrow[row_slot] = np.where(valid, np.float16(0.0), np.float16(MASKV))
    msk = np.ascontiguousarray(
        mrow.reshape(NT, 128, D_PAD).transpose(1, 0, 2).reshape(128, NT * D_PAD))

    # per-row node-column one-hot [128, NT*128] f16 (host-built, DMA'd in)
    crow_slot = np.zeros(rows_total, np.int32)
    crow_slot[row_slot] = np.repeat(p["crow_of"], r_n).astype(np.int32)
    ohx = (crow_slot.reshape(NT, 128)[:, :, None]
           == np.arange(128, dtype=np.int32)[None, None, :])
    oh = np.ascontiguousarray(
        ohx.transpose(1, 0, 2).reshape(128, NT * 128).astype(np.float16))

    # one merged DMA block per supertile: [kv 8192 | oh 1024 | msk 64]
    ST = NT // TC
    datx = np.empty((128, ST, SBLK), np.float16)
    datx[:, :, 0:TC * 1024] = kvx.reshape(128, ST, TC * 1024)
    datx[:, :, TC * 1024:TC * 1152] = oh.reshape(128, ST, TC * 128)
    datx[:, :, TC * 1152:SBLK] = msk.reshape(128, ST, TC * D_PAD)

    return dict(datx=np.ascontiguousarray(datx.reshape(128, ST * SBLK)), qx=qx)


def _build_program(NT):
    import concourse.bass as bass
    import concourse.tile as tile
    from concourse import bacc, mybir

    f32 = mybir.dt.float32
    f16 = mybir.dt.float16
    AO = mybir.AluOpType
    ST = NT // TC

    nc = bacc.Bacc("TRN2", target_bir_lowering=False, debug=False,
                   num_devices=NCORES)

    datxd = nc.dram_tensor("datx", [128, (NT // TC) * SBLK], f16,
                           kind="ExternalInput").ap()
    qx = nc.dram_tensor("qx", [65, NT * 128], f16, kind="ExternalInput").ap()
    wkvd = nc.dram_tensor("wkv", [128, 128], f16, kind="ExternalInput").ap()
    wqd = nc.dram_tensor("wq", [65, DIM], f16, kind="ExternalInput").ap()
    wo2d = nc.dram_tensor("wo2", [128, 128], f16, kind="ExternalInput").ap()
    bo2d = nc.dram_tensor("bo2", [128, 1], f32, kind="ExternalInput").ap()
    comb = nc.dram_tensor("comb", [128, (NT // 2) * 128], f32,
                          kind="ExternalOutput").ap()

    def apx(t, dims, extra_off=0):
        a = t[:]
        return bass.AP(a.tensor, a.offset + extra_off, [list(a.ap[0])] + dims)

    with tile.TileContext(nc) as tc, ExitStack() as ctx, \
            nc.allow_low_precision("fp16 edge softmax within 2e-2 tolerance"):
        consts = ctx.enter_context(tc.tile_pool(name="consts", bufs=1))
        ld = ctx.enter_context(tc.tile_pool(name="ld", bufs=4))
        work = ctx.enter_context(tc.tile_pool(name="work", bufs=4))
        qw = ctx.enter_context(tc.tile_pool(name="qw", bufs=3))
        scw = ctx.enter_context(tc.tile_pool(name="scw", bufs=6))
        adexp = ctx.enter_context(tc.tile_pool(name="adexp", bufs=2 * NP + 4))
        pstp = ctx.enter_context(tc.tile_pool(name="pstp", bufs=2, space="PSUM"))
        qpsp = ctx.enter_context(tc.tile_pool(name="qpsp", bufs=2, space="PSUM"))
        cpsp = ctx.enter_context(tc.tile_pool(name="cpsp", bufs=2, space="PSUM"))

        from concourse.masks import make_identity

        wkv_sb = consts.tile([128, 128], f16)
        nc.sync.dma_start(wkv_sb[:], wkvd[:, :])
        wq_sb = consts.tile([65, DIM], f16)
        nc.sync.dma_start(wq_sb[:], wqd[:, :])
        wo2_sb = consts.tile([128, 128], f16)
        nc.sync.dma_start(wo2_sb[:], wo2d[:, :])
        bo2_sb = consts.tile([128, 1], f32)
        nc.sync.dma_start(bo2_sb[:], bo2d[:, :])
        ident = consts.tile([128, 128], f16)
        make_identity(nc, ident[:])
        esh = consts.tile([128, 1], f32)
        nc.vector.memset(esh[:], ESHIFT)

        prev = None

        def emit_loads(st):
            datld = ld.tile([128, SBLK], f16, tag="datld")
            nc.sync.dma_start(datld[:], datxd[:, st * SBLK:(st + 1) * SBLK])
            qld = ld.tile([65, TC * 128], f16, tag="qld")
            nc.sync.dma_start(qld[:], qx[:, st * TC * 128:(st + 1) * TC * 128])
            return datld, qld

        def emit_A(st, datld, qld):
            kvld = datld  # kv cols [0 : TC*1024]
            OHOFF = TC * 1024
            MOFF = TC * 1152

            # q~ projection, all TC tiles into one PSUM bank, one f16 copy-out
            qps = qpsp.tile([128, TC, DIM], f32, space="PSUM", tag="qps")
            for t in range(TC):
                nc.tensor.matmul(out=qps[:, t, :],
                                 lhsT=qld[:, t * 128:(t + 1) * 128],
                                 rhs=wq_sb[:], start=True, stop=True)
            q16 = qw.tile([128, TC * DIM], f16, tag="q16")
            nc.scalar.copy(q16[:], qps[:])

            adex_l = []
            for u in range(NP):
                sco = scw.tile([128, 2, D_PAD, H], f16, tag="sco")
                adex = adexp.tile([128, 2, D_PAD, DIM + H], f16, tag="adex")
                vtp = work.tile([128, 2, D_PAD * DIM], f16, tag="vt16")
                prodp = work.tile([128, 2, D_PAD, DIM], f16, tag="prod")
                for tp in range(2):
                    t = 2 * u + tp
                    # per-edge [k^ || v^] projection: 8 slots -> PSUM
                    pst = pstp.tile([128, D_PAD, 128], f32, space="PSUM",
                                    tag="pst")
                    for sl in range(D_PAD):
                        nc.tensor.matmul(
                            out=pst[:, sl, :],
                            lhsT=kvld[:, (t * D_PAD + sl) * 128:
                                      (t * D_PAD + sl + 1) * 128],
                            rhs=wkv_sb[:], start=True, stop=True)
                    # scores: prod = k^ * q~ (slot-bcast)
                    nc.vector.tensor_tensor(
                        out=prodp[:, tp],
                        in0=apx(pst, [[128, D_PAD], [1, DIM]]),
                        in1=apx(q16, [[0, D_PAD], [1, DIM]], extra_off=t * DIM),
                        op=AO.mult)
                    # v^ to SBUF f16 for the GpSimd weighting
                    nc.scalar.copy(
                        vtp[:, tp], apx(pst, [[128, D_PAD], [1, DIM]],
                                        extra_off=DIM))
                # one per-head reduce for the whole pair
                nc.vector.tensor_reduce(
                    out=sco[:],
                    in_=apx(prodp, [[DK, 2 * D_PAD * H], [1, DK]]),
                    axis=mybir.AxisListType.X, op=AO.add)
                # mask both tiles at once (GpSimd: SBUF-only op, DVE is hot)
                nc.gpsimd.tensor_tensor(
                    out=sco[:], in0=sco[:],
                    in1=apx(datld, [[1, 2 * D_PAD], [0, H]],
                            extra_off=MOFF + 2 * u * D_PAD),
                    op=AO.add)
                # exp for both tiles straight into adex cols [64:68]
                nc.scalar.activation(
                    out=apx(adex, [[DIM + H, 2 * D_PAD], [1, H]],
                            extra_off=DIM),
                    in_=sco[:],
                    func=mybir.ActivationFunctionType.Exp,
                    scale=1.0 / np.sqrt(DK), bias=esh[:])
                # adex[., 0:64] = v^ * exp for both tiles in one op
                nc.gpsimd.tensor_tensor(
                    out=apx(adex, [[(DIM + H) * D_PAD, 2], [DIM + H, D_PAD],
                                   [1, DIM]]),
                    in0=vtp[:],
                    in1=apx(adex, [[(DIM + H) * D_PAD, 2], [DIM + H, D_PAD],
                                   [1, H], [0, DK]], extra_off=DIM),
                    op=AO.mult)
                adex_l.append(adex)
            return dict(st=st, adex=adex_l, datld=datld)

        def emit_B(state):
            st = state["st"]
            datld = state["datld"]
            OHOFF = TC * 1024
            osbq = scw.tile([128, NP * 128], f32, tag="osbq")

            def b_comb(u):
                adex = state["adex"][u]
                # one PSUM bank per pair: f32 [0:136] = per-tile combines,
                # f16 elems [272:400] = transpose out, f32 [200:328] = Wo out
                mega = cpsp.tile([128, 512], f32, space="PSUM", tag="mega")
                for tp in range(2):
                    t = 2 * u + tp
                    cp = mega[:, tp * 68:(tp + 1) * 68]
                    for sl in range(D_PAD):
                        nc.tensor.matmul(
                            out=cp,
                            lhsT=datld[:, OHOFF + t * 128:OHOFF + (t + 1) * 128],
                            rhs=adex[:, tp, sl, :],
                            start=(sl == 0), stop=(sl == D_PAD - 1))
                return mega

            def b_tail(u, mega):
                meg16 = mega.bitcast(f16)
                rd = scw.tile([128, 2, H], f32, tag="rd")
                nc.vector.reciprocal(
                    rd[:], apx(mega, [[68, 2], [1, H]], extra_off=DIM))
                # nrm = min(rd, 1e7) * cp in one fused op; the clamp keeps
                # empty node columns (den=0, rd=inf) at 0*big=0 -- the
                # block-diag Wo matmul would spread 0*inf=NaN otherwise
                nrm2 = scw.tile([128, 2, DIM], f16, tag="nrm2")
                nc.vector.scalar_tensor_tensor(
                    out=nrm2[:],
                    in0=apx(rd, [[H, 2], [1, H], [0, DK]]),
                    scalar=1.0e7,
                    in1=apx(mega, [[68, 2], [1, DIM]]),
                    op0=AO.min,
                    op1=AO.mult)
                # transpose both tiles at once, then block-diag Wo matmul
                tps = meg16[:, 272:400]
                nc.tensor.transpose(out=tps, in_=apx(nrm2, [[1, 128]]),
                                    identity=ident[:])
                nrmT2 = scw.tile([128, 128], f16, tag="nrmT2")
                nc.scalar.copy(nrmT2[:], tps)
                wout = mega[:, 200:328]
                nc.tensor.matmul(out=wout, lhsT=wo2_sb[:], rhs=nrmT2[:],
                                 start=True, stop=True)
                # bias-add fused into the PSUM->SBUF copy
                nc.scalar.activation(
                    out=osbq[:, u * 128:(u + 1) * 128], in_=wout,
                    func=mybir.ActivationFunctionType.Identity,
                    bias=bo2_sb[:], scale=1.0)

            # pairwise: two combines ahead, so the PE's DVE round-trip wait
            # hides under the following combine
            megas = {}
            megas[0] = b_comb(0)
            megas[1] = b_comb(1)
            b_tail(0, megas[0])
            megas[2] = b_comb(2)
            b_tail(1, megas[1])
            megas[3] = b_comb(3)
            b_tail(2, megas[2])
            b_tail(3, megas[3])
            # one batched store per supertile (sync queue, after prefetches)
            nc.sync.dma_start(
                comb[:, st * NP * 128:(st + 1) * NP * 128], osbq[:])

        loads = {}
        loads[0] = emit_loads(0)
        if ST > 1:
            loads[1] = emit_loads(1)
        for st in range(ST):
            if st + 2 < ST:
                loads[st + 2] = emit_loads(st + 2)
            state = emit_A(st, *loads.pop(st))
            if prev is not None:
                emit_B(prev)
            prev = state
        emit_B(prev)

    nc.compile()
    return nc


def kernel(**inputs):
    from concourse.bass_utils import run_bass_kernel_spmd

    query = np.asarray(inputs["query"], np.float32)
    key = np.asarray(inputs["key"], np.float32)
    value = np.asarray(inputs["value"], np.float32)
    src = np.asarray(inputs["src"])
    dst = np.asarray(inputs["dst"])
    Wq = np.asarray(inputs["Wq"], np.float32)
    bq = np.asarray(inputs["bq"], np.float32)
    Wk = np.asarray(inputs["Wk"], np.float32)
    bk = np.asarray(inputs["bk"], np.float32)  # noqa: F841  (cancels in softmax)
    Wv = np.asarray(inputs["Wv"], np.float32)
    bv = np.asarray(inputs["bv"], np.float32)
    Wo = np.asarray(inputs["Wo"], np.float32)
    bo = np.asarray(inputs["bo"], np.float32)

    packs, ssrc, cum, NT = _host_prep(src, dst)
    nc = _build_program(NT)

    keyT16 = np.ascontiguousarray(key.T).astype(np.float16)
    valT16 = np.ascontiguousarray(value.T).astype(np.float16)
    qT16 = np.ascontiguousarray(query.T).astype(np.float16)

    # weight packing with bias folding (the q.bk score term cancels in the
    # per-segment softmax and is dropped; bv folds into bo')
    wkv = np.zeros((128, 128), np.float16)
    wkv[0:64, 0:64] = Wk.T
    wkv[64:128, 64:128] = Wv.T
    wq = np.zeros((65, DIM), np.float16)
    wq[0:64, :] = Wq.T
    wq[64, :] = bq
    wo2 = np.zeros((128, 128), np.float16)
    wo2[0:64, 0:64] = Wo.T
    wo2[64:128, 64:128] = Wo.T
    bo_eff = (bo + Wo @ bv).astype(np.float32)
    bo2 = np.concatenate([bo_eff, bo_eff]).reshape(128, 1).astype(np.float32)

    in_maps = []
    for p in packs:
        ex = _expand_core(p, ssrc, cum, NT, keyT16, valT16, qT16)
        in_maps.append(dict(datx=ex["datx"], qx=ex["qx"],
                            wkv=wkv, wq=wq, wo2=wo2, bo2=bo2))

    trace = bool(int(os.environ.get("KERNEL_TRACE", "0")))
    res = run_bass_kernel_spmd(
        nc, in_maps, core_ids=list(range(NCORES)), trace=trace,
        tmpdir=os.environ.get("KERNEL_TRACE_DIR") or None,
    )
    kernel.last_results = res

    out = np.empty((N, DIM), np.float32)
    for p, r in zip(packs, res.results):
        cols = (p["tile_of"] // 2) * 128 + p["crow_of"]
        half = (p["tile_of"] % 2).astype(bool)
        sel = r["comb"][:, cols]  # [128, nn]
        out[p["n0"]:p["n1"]] = np.where(half[:, None], sel[64:128, :].T,
                                        sel[0:64, :].T)
        z = p["d"] == 0
        if z.any():
            out[p["n0"]:p["n1"]][z] = bo
    return out


# revision 26
# speedup vs baseline: 1.1584x; 1.0052x over previous
"""Graph multi-head attention (GNN message passing) on 8 Trainium2 NeuronCores.

Strategy (dst-sharded edge parallelism, zero indirect DMAs):
  - Host: sort edges by dst, split nodes into 8 contiguous ranges with ~equal
    edge counts. Each core owns all incoming edges of its node range, so the
    per-dst segment softmax is core-local.
  - Host EXPANDS the raw per-edge operands: for every packed edge slot the
    fp16 [key||value] column of its src node, and per virtual row the fp16
    query column of its dst node, plus the per-tile one-hot combine matrix
    and the slot-validity mask. The device projects k/q/v per edge with
    plain matmuls -- every DMA is a large contiguous load. kv, one-hot, and
    mask are interleaved into ONE dram block per supertile (single load),
    prefetched two supertiles ahead on the sync queue; outputs are staged
    to one batched store per supertile so stores never gate load prefetch.
  - Edges are packed into fixed-width virtual rows (node, up to D_PAD=8
    incoming edges); rows of one node stay inside one 128-row tile and are
    combined with a host-supplied one-hot matmul, PSUM-accumulated over all
    8 slots (denominator columns ride along), then normalized on DVE.
  - Bias folding: bq via a ones row in the q expansion + [Wq.T; bq] rhs;
    bv folds into bo' = bo + Wo bv (sum(alpha)=1); the q.bk score term is
    DROPPED ENTIRELY -- it is constant across all edges of a (dst, head)
    segment, so it cancels in the segment softmax (exact).
  - Engine balance (DVE is the critical engine): scores via one PSUM-read
    multiply + one pair-level reduce on DVE; mask-add on GpSimd; exp on ACT
    (one op per tile pair, written straight into the combine operand); the
    alpha*v weighting on GpSimd from an ACT-copied f16 v^; normalize via a
    fused (min 1e7)(recip)*cp scalar_tensor_tensor (the clamp keeps empty
    node columns at 0 instead of 0*inf=NaN, which the block-diag Wo matmul
    would otherwise spread); q projections batched per supertile.
  - v stays in HEAD space through aggregation; tiles are processed in
    PAIRS: one 128x128 PE transpose per pair, then a single block-diagonal
    [[Wo.T,0],[0,Wo.T]] matmul projects both tiles, and one ACT copy with
    per-partition bias adds bo' for both. Output is stored feature-major
    ([128, NPAIR*128]); host unshards with fancy indexing.
  - Segment-max subtraction is replaced by a constant exp shift
    (exp(s/4 - 8)); invalid slots get an additive -30000 fp16 mask.
    Degree-0 nodes are fixed up to `bo` on the host.
"""

import os
from contextlib import ExitStack

import numpy as np

N = 100000
E = 1600000
DIM = 64
H = 4
DK = DIM // H
NCORES = 8

D_PAD = 8          # edge slots per virtual row
TC = 8             # 128-row tiles per supertile
NP = TC // 2       # tile pairs per supertile
SBLK = TC * 1024 + TC * 128 + TC * D_PAD  # merged [kv|oh|msk] block cols
MASKV = -30000.0   # additive fp16-safe -inf
ESHIFT = -8.0      # constant exp shift: keeps exp() in fp16 range both ways


def _host_prep(src, dst):
    """Pack edges into per-core tiling metadata (no feature expansion yet)."""
    src = np.asarray(src).astype(np.int64)
    dst = np.asarray(dst).astype(np.int64)
    order = np.argsort(dst, kind="stable")
    ssrc = src[order]
    deg = np.bincount(dst, minlength=N).astype(np.int64)
    cum = np.concatenate([[0], np.cumsum(deg)])

    bounds = [0]
    for c in range(1, NCORES):
        t = round(c * E / NCORES)
        n = int(np.searchsorted(cum, t, side="left"))
        n = min(max(n, bounds[-1] + 1), N - (NCORES - c))
        bounds.append(n)
    bounds.append(N)

    packs = []
    for c in range(NCORES):
        n0, n1 = bounds[c], bounds[c + 1]
        nn = n1 - n0
        d = deg[n0:n1]
        r_n = np.maximum(1, -(-d // D_PAD)).astype(np.int64)
        tile_of = np.empty(nn, np.int64)
        colrow = np.empty(nn, np.int64)
        crow_of = np.empty(nn, np.int64)
        t_id = 0
        rows_in = 0
        nodes_in = 0
        for i in range(nn):
            r = r_n[i]
            if rows_in + r > 128:
                t_id += 1
                rows_in = 0
                nodes_in = 0
            tile_of[i] = t_id
            colrow[i] = rows_in
            crow_of[i] = nodes_in
            rows_in += r
            nodes_in += 1
        packs.append(dict(n0=n0, n1=n1, nn=nn, d=d, r_n=r_n, tile_of=tile_of,
                          colrow=colrow, crow_of=crow_of, nt=t_id + 1))

    NT = -(-max(p["nt"] for p in packs) // TC) * TC
    return packs, ssrc, cum, NT


def _expand_core(p, ssrc, cum, NT, keyT16, valT16, qT16):
    """Build the per-core expanded fp16 operand arrays."""
    n0 = p["n0"]
    nn = p["nn"]
    d, r_n = p["d"], p["r_n"]
    rows_total = NT * 128

    row_node = np.repeat(np.arange(nn), r_n)
    starts = np.concatenate([[0], np.cumsum(r_n)])[:-1]
    row_k = np.arange(len(row_node)) - np.repeat(starts, r_n)
    row_slot = (np.repeat(p["tile_of"], r_n) * 128
                + np.repeat(p["colrow"], r_n) + row_k)
    row_deg = np.clip(np.repeat(d, r_n) - row_k * D_PAD, 0, D_PAD)
    row_e0 = cum[n0 + row_node] + row_k * D_PAD
    j = np.arange(D_PAD)[None, :]
    valid = j < row_deg[:, None]
    eidx = np.minimum(row_e0[:, None] + j, E - 1)
    srcv = ssrc[eidx]

    # kvx[0:64, col]=key.T[src], [64:128]=value.T[src]; col=T*1024+s*128+p
    kvx = np.zeros((128, NT * 1024), np.float16)
    T_of = row_slot // 128
    p_of = row_slot % 128
    cols = T_of[:, None] * 1024 + j * 128 + p_of[:, None]
    cv = cols[valid]
    sv = srcv[valid]
    kvx[0:64, cv] = keyT16[:, sv]
    kvx[64:128, cv] = valT16[:, sv]

    # qx [65, NT*128], ones row for bq folding
    qx = np.zeros((65, NT * 128), np.float16)
    qx[64, :] = 1.0
    qx[0:64, row_slot] = qT16[:, n0 + row_node]

    # additive mask [128, NT*8], col = T*8 + s (no head replication)
    mrow = np.full((rows_total, D_PAD), MASKV, np.float16)
    mrow[row_slot] = np.where(valid, np.float16(0.0), np.float16(MASKV))
    msk = np.ascontiguousarray(
        mrow.reshape(NT, 128, D_PAD).transpose(1, 0, 2).reshape(128, NT * D_PAD))

    # per-row node-column one-hot [128, NT*128] f16 (host-built, DMA'd in)
    crow_slot = np.zeros(rows_total, np.int32)
    crow_slot[row_slot] = np.repeat(p["crow_of"], r_n).astype(np.int32)
    ohx = (crow_slot.reshape(NT, 128)[:, :, None]
           == np.arange(128, dtype=np.int32)[None, None, :])
    oh = np.ascontiguousarray(
        ohx.transpose(1, 0, 2).reshape(128, NT * 128).astype(np.float16))

    # one merged DMA block per supertile: [kv 8192 | oh 1024 | msk 64]
    ST = NT // TC
    datx = np.empty((128, ST, SBLK), np.float16)
    datx[:, :, 0:TC * 1024] = kvx.reshape(128, ST, TC * 1024)
    datx[:, :, TC * 1024:TC * 1152] = oh.reshape(128, ST, TC * 128)
    datx[:, :, TC * 1152:SBLK] = msk.reshape(128, ST, TC * D_PAD)

    return dict(datx=np.ascontiguousarray(datx.reshape(128, ST * SBLK)), qx=qx)


def _build_program(NT):
    import concourse.bass as bass
    import concourse.tile as tile
    from concourse import bacc, mybir

    f32 = mybir.dt.float32
    f16 = mybir.dt.float16
    AO = mybir.AluOpType
    ST = NT // TC

    nc = bacc.Bacc("TRN2", target_bir_lowering=False, debug=False,
                   num_devices=NCORES)

    datxd = nc.dram_tensor("datx", [128, (NT // TC) * SBLK], f16,
                           kind="ExternalInput").ap()
    qx = nc.dram_tensor("qx", [65, NT * 128], f16, kind="ExternalInput").ap()
    wkvd = nc.dram_tensor("wkv", [128, 128], f16, kind="ExternalInput").ap()
    wqd = nc.dram_tensor("wq", [65, DIM], f16, kind="ExternalInput").ap()
    wo2d = nc.dram_tensor("wo2", [128, 128], f16, kind="ExternalInput").ap()
    bo2d = nc.dram_tensor("bo2", [128, 1], f32, kind="ExternalInput").ap()
    comb = nc.dram_tensor("comb", [128, (NT // 2) * 128], f32,
                          kind="ExternalOutput").ap()

    def apx(t, dims, extra_off=0):
        a = t[:]
        return bass.AP(a.tensor, a.offset + extra_off, [list(a.ap[0])] + dims)

    with tile.TileContext(nc) as tc, ExitStack() as ctx, \
            nc.allow_low_precision("fp16 edge softmax within 2e-2 tolerance"):
        consts = ctx.enter_context(tc.tile_pool(name="consts", bufs=1))
        ld = ctx.enter_context(tc.tile_pool(name="ld", bufs=4))
        work = ctx.enter_context(tc.tile_pool(name="work", bufs=4))
        qw = ctx.enter_context(tc.tile_pool(name="qw", bufs=3))
        scw = ctx.enter_context(tc.tile_pool(name="scw", bufs=6))
        adexp = ctx.enter_context(tc.tile_pool(name="adexp", bufs=2 * NP + 4))
        pstp = ctx.enter_context(tc.tile_pool(name="pstp", bufs=2, space="PSUM"))
        qpsp = ctx.enter_context(tc.tile_pool(name="qpsp", bufs=2, space="PSUM"))
        cpsp = ctx.enter_context(tc.tile_pool(name="cpsp", bufs=2, space="PSUM"))

        from concourse.masks import make_identity

        wkv_sb = consts.tile([128, 128], f16)
        nc.sync.dma_start(wkv_sb[:], wkvd[:, :])
        wq_sb = consts.tile([65, DIM], f16)
        nc.sync.dma_start(wq_sb[:], wqd[:, :])
        wo2_sb = consts.tile([128, 128], f16)
        nc.sync.dma_start(wo2_sb[:], wo2d[:, :])
        bo2_sb = consts.tile([128, 1], f32)
        nc.sync.dma_start(bo2_sb[:], bo2d[:, :])
        ident = consts.tile([128, 128], f16)
        make_identity(nc, ident[:])
        esh = consts.tile([128, 1], f32)
        nc.vector.memset(esh[:], ESHIFT)

        prev = None

        def emit_loads(st):
            datld = ld.tile([128, SBLK], f16, tag="datld")
            nc.sync.dma_start(datld[:], datxd[:, st * SBLK:(st + 1) * SBLK])
            qld = ld.tile([65, TC * 128], f16, tag="qld")
            nc.sync.dma_start(qld[:], qx[:, st * TC * 128:(st + 1) * TC * 128])
            return datld, qld

        def emit_A_head(st, datld, qld):
            # q~ projection, all TC tiles into one PSUM bank, one f16 copy-out
            qps = qpsp.tile([128, TC, DIM], f32, space="PSUM", tag="qps")
            for t in range(TC):
                nc.tensor.matmul(out=qps[:, t, :],
                                 lhsT=qld[:, t * 128:(t + 1) * 128],
                                 rhs=wq_sb[:], start=True, stop=True)
            q16 = qw.tile([128, TC * DIM], f16, tag="q16")
            nc.scalar.copy(q16[:], qps[:])
            return q16

        def emit_A_pair(st, u, datld, q16):
            kvld = datld  # kv cols [0 : TC*1024]
            MOFF = TC * 1152
            if True:
                sco = scw.tile([128, 2, D_PAD, H], f16, tag="sco")
                adex = adexp.tile([128, 2, D_PAD, DIM + H], f16, tag="adex")
                vtp = work.tile([128, 2, D_PAD * DIM], f16, tag="vt16")
                prodp = work.tile([128, 2, D_PAD, DIM], f16, tag="prod")
                for tp in range(2):
                    t = 2 * u + tp
                    # per-edge [k^ || v^] projection: 8 slots -> PSUM
                    pst = pstp.tile([128, D_PAD, 128], f32, space="PSUM",
                                    tag="pst")
                    for sl in range(D_PAD):
                        nc.tensor.matmul(
                            out=pst[:, sl, :],
                            lhsT=kvld[:, (t * D_PAD + sl) * 128:
                                      (t * D_PAD + sl + 1) * 128],
                            rhs=wkv_sb[:], start=True, stop=True)
                    # scores: prod = k^ * q~ (slot-bcast)
                    nc.vector.tensor_tensor(
                        out=prodp[:, tp],
                        in0=apx(pst, [[128, D_PAD], [1, DIM]]),
                        in1=apx(q16, [[0, D_PAD], [1, DIM]], extra_off=t * DIM),
                        op=AO.mult)
                    # v^ to SBUF f16 for the GpSimd weighting
                    nc.scalar.copy(
                        vtp[:, tp], apx(pst, [[128, D_PAD], [1, DIM]],
                                        extra_off=DIM))
                # one per-head reduce for the whole pair
                nc.vector.tensor_reduce(
                    out=sco[:],
                    in_=apx(prodp, [[DK, 2 * D_PAD * H], [1, DK]]),
                    axis=mybir.AxisListType.X, op=AO.add)
                # mask both tiles at once (GpSimd: SBUF-only op, DVE is hot)
                nc.gpsimd.tensor_tensor(
                    out=sco[:], in0=sco[:],
                    in1=apx(datld, [[1, 2 * D_PAD], [0, H]],
                            extra_off=MOFF + 2 * u * D_PAD),
                    op=AO.add)
                # exp for both tiles straight into adex cols [64:68]
                nc.scalar.activation(
                    out=apx(adex, [[DIM + H, 2 * D_PAD], [1, H]],
                            extra_off=DIM),
                    in_=sco[:],
                    func=mybir.ActivationFunctionType.Exp,
                    scale=1.0 / np.sqrt(DK), bias=esh[:])
                # adex[., 0:64] = v^ * exp for both tiles in one op
                nc.gpsimd.tensor_tensor(
                    out=apx(adex, [[(DIM + H) * D_PAD, 2], [DIM + H, D_PAD],
                                   [1, DIM]]),
                    in0=vtp[:],
                    in1=apx(adex, [[(DIM + H) * D_PAD, 2], [DIM + H, D_PAD],
                                   [1, H], [0, DK]], extra_off=DIM),
                    op=AO.mult)
            return adex

        def b_comb(state, u):
            datld = state["datld"]
            OHOFF = TC * 1024
            adex = state["adex"][u]
            # one PSUM bank per pair: f32 [0:136] = per-tile combines,
            # f16 elems [272:400] = transpose out, f32 [200:328] = Wo out
            mega = cpsp.tile([128, 512], f32, space="PSUM", tag="mega")
            for tp in range(2):
                t = 2 * u + tp
                cp = mega[:, tp * 68:(tp + 1) * 68]
                for sl in range(D_PAD):
                    nc.tensor.matmul(
                        out=cp,
                        lhsT=datld[:, OHOFF + t * 128:OHOFF + (t + 1) * 128],
                        rhs=adex[:, tp, sl, :],
                        start=(sl == 0), stop=(sl == D_PAD - 1))
            return mega

        def emit_B_tail(state, u, mega):
            osbq = state["osbq"]
            if True:
                meg16 = mega.bitcast(f16)
                rd = scw.tile([128, 2, H], f32, tag="rd")
                nc.vector.reciprocal(
                    rd[:], apx(mega, [[68, 2], [1, H]], extra_off=DIM))
                # nrm = min(rd, 1e7) * cp in one fused op; the clamp keeps
                # empty node columns (den=0, rd=inf) at 0*big=0 -- the
                # block-diag Wo matmul would spread 0*inf=NaN otherwise
                nrm2 = scw.tile([128, 2, DIM], f16, tag="nrm2")
                nc.vector.scalar_tensor_tensor(
                    out=nrm2[:],
                    in0=apx(rd, [[H, 2], [1, H], [0, DK]]),
                    scalar=1.0e7,
                    in1=apx(mega, [[68, 2], [1, DIM]]),
                    op0=AO.min,
                    op1=AO.mult)
                # transpose both tiles at once, then block-diag Wo matmul
                tps = meg16[:, 272:400]
                nc.tensor.transpose(out=tps, in_=apx(nrm2, [[1, 128]]),
                                    identity=ident[:])
                nrmT2 = scw.tile([128, 128], f16, tag="nrmT2")
                nc.scalar.copy(nrmT2[:], tps)
                wout = mega[:, 200:328]
                nc.tensor.matmul(out=wout, lhsT=wo2_sb[:], rhs=nrmT2[:],
                                 start=True, stop=True)
                # bias-add fused into the PSUM->SBUF copy
                nc.scalar.activation(
                    out=osbq[:, u * 128:(u + 1) * 128], in_=wout,
                    func=mybir.ActivationFunctionType.Identity,
                    bias=bo2_sb[:], scale=1.0)

        def emit_iter(st, prev):
            # A(st) fully, then B(prev) with two combines emitted ahead
            if st is not None:
                datld, qld = loads.pop(st)
                q16 = emit_A_head(st, datld, qld)
                adex_l = [emit_A_pair(st, u, datld, q16) for u in range(NP)]
            if prev is not None:
                osbq = scw.tile([128, NP * 128], f32, tag="osbq")
                prev["osbq"] = osbq
                megas = {}
                megas[0] = b_comb(prev, 0)
                megas[1] = b_comb(prev, 1)
                emit_B_tail(prev, 0, megas[0])
                megas[2] = b_comb(prev, 2)
                emit_B_tail(prev, 1, megas[1])
                megas[3] = b_comb(prev, 3)
                emit_B_tail(prev, 2, megas[2])
                emit_B_tail(prev, 3, megas[3])
                nc.sync.dma_start(
                    comb[:, prev["st"] * NP * 128:
                         (prev["st"] + 1) * NP * 128], prev["osbq"][:])
            if st is not None:
                return dict(st=st, adex=adex_l, datld=datld)
            return None

        loads = {}
        loads[0] = emit_loads(0)
        if ST > 1:
            loads[1] = emit_loads(1)
        for st in range(ST):
            if st + 2 < ST:
                loads[st + 2] = emit_loads(st + 2)
            prev = emit_iter(st, prev)
        emit_iter(None, prev)

    nc.compile()
    return nc


def kernel(**inputs):
    from concourse.bass_utils import run_bass_kernel_spmd

    query = np.asarray(inputs["query"], np.float32)
    key = np.asarray(inputs["key"], np.float32)
    value = np.asarray(inputs["value"], np.float32)
    src = np.asarray(inputs["src"])
    dst = np.asarray(inputs["dst"])
    Wq = np.asarray(inputs["Wq"], np.float32)
    bq = np.asarray(inputs["bq"], np.float32)
    Wk = np.asarray(inputs["Wk"], np.float32)
    bk = np.asarray(inputs["bk"], np.float32)  # noqa: F841  (cancels in softmax)
    Wv = np.asarray(inputs["Wv"], np.float32)
    bv = np.asarray(inputs["bv"], np.float32)
    Wo = np.asarray(inputs["Wo"], np.float32)
    bo = np.asarray(inputs["bo"], np.float32)

    packs, ssrc, cum, NT = _host_prep(src, dst)
    nc = _build_program(NT)

    keyT16 = np.ascontiguousarray(key.T).astype(np.float16)
    valT16 = np.ascontiguousarray(value.T).astype(np.float16)
    qT16 = np.ascontiguousarray(query.T).astype(np.float16)

    # weight packing with bias folding (the q.bk score term cancels in the
    # per-segment softmax and is dropped; bv folds into bo')
    wkv = np.zeros((128, 128), np.float16)
    wkv[0:64, 0:64] = Wk.T
    wkv[64:128, 64:128] = Wv.T
    wq = np.zeros((65, DIM), np.float16)
    wq[0:64, :] = Wq.T
    wq[64, :] = bq
    wo2 = np.zeros((128, 128), np.float16)
    wo2[0:64, 0:64] = Wo.T
    wo2[64:128, 64:128] = Wo.T
    bo_eff = (bo + Wo @ bv).astype(np.float32)
    bo2 = np.concatenate([bo_eff, bo_eff]).reshape(128, 1).astype(np.float32)

    in_maps = []
    for p in packs:
        ex = _expand_core(p, ssrc, cum, NT, keyT16, valT16, qT16)
        in_maps.append(dict(datx=ex["datx"], qx=ex["qx"],
                            wkv=wkv, wq=wq, wo2=wo2, bo2=bo2))

    trace = bool(int(os.environ.get("KERNEL_TRACE", "0")))
    res = run_bass_kernel_spmd(
        nc, in_maps, core_ids=list(range(NCORES)), trace=trace,
        tmpdir=os.environ.get("KERNEL_TRACE_DIR") or None,
    )
    kernel.last_results = res

    out = np.empty((N, DIM), np.float32)
    for p, r in zip(packs, res.results):
        cols = (p["tile_of"] // 2) * 128 + p["crow_of"]
        half = (p["tile_of"] % 2).astype(bool)
        sel = r["comb"][:, cols]  # [128, nn]
        out[p["n0"]:p["n1"]] = np.where(half[:, None], sel[64:128, :].T,
                                        sel[0:64, :].T)
        z = p["d"] == 0
        if z.any():
            out[p["n0"]:p["n1"]][z] = bo
    return out
